# revision 21
# baseline (speedup 1.0000x reference)
"""Causal self-attention (B=2, T=2048, D=1024, H=16) on 8 TRN2 NeuronCores.

Sharding: data-parallel over batch (2) x tensor-parallel over head groups (4),
so each core handles one batch element and 4 heads (256 of the 1024 attention
channels). The out-projection is row-sharded; the host sums the 4 partial
outputs per batch element in fp32.

v3 schedule (v1 baseline ~190us, v2 ~234us):
  - one consolidated DMA descriptor per logical input block (9 total) in
    token-chunk-major order, so the DMA-issue queue (~0.6us per descriptor)
    never gates data arrival; packets of one descriptor spread across all
    16 DMA engines
  - ~12 dummy matmuls at t=0 keep the PE busy through the DMA fill so the
    HAM clock-gate reaches K=8/8 (2.4 GHz) before real work starts
  - all projection work not needed up front is wrapped in generators and
    pumped between the S^T->exp->PV stages of each attention key-group, so
    the PE always has independent work while ScalarE (exp, the attention
    pacer at ~2.25us/key-group) runs; out-projection tiles are pumped the
    same way during the second head-pair's strips instead of as a tail
  - causal narrowing: S^T, exp and PV skip the fully-masked query range of
    diagonal key tiles (query quantization 128); only the within-block
    triangle is masked, with a [128,128] affine_select per diagonal block
  - softmax denominator comes free as PV row 64 (lhsT = [V | 1]); the
    1/l broadcast across the 64 output channels is a K=1 PE matmul into a
    reused PSUM bank + one DVE reciprocal -- no DRAM round trip
Bias handling: b_k dropped (softmax shift-invariant per query), b_q applied
via a DVE per-partition scalar add, b_v and b_out folded into a host-side
constant (softmax rows sum to 1).
"""

import numpy as np

B, T_FULL, D, H = 2, 2048, 1024, 16
DH = 64
HC = 4            # heads per core
OC = HC * DH      # 256 attention channels per core
NCORES = 8

NDUMMY = 8        # PE warm-up matmuls during the initial DMA fill
PUMP_N = 4        # filler units (~2 matmuls each) emitted per key-group


def build_nc(T=T_FULL):
    import concourse.bass as bass
    import concourse.mybir as mybir
    from concourse import bacc
    from concourse.tile import TileContext

    f32 = mybir.dt.float32
    f32r = mybir.dt.float32r
    fp16 = mybir.dt.float16
    AF = mybir.ActivationFunctionType
    ALU = mybir.AluOpType

    def mm(out, lhsT, rhs, start, stop, **kw):
        if lhsT.dtype == f32:
            lhsT = lhsT.bitcast(f32r)
        if rhs.dtype == f32:
            rhs = rhs.bitcast(f32r)
        nc.tensor.matmul(out, lhsT, rhs, start=start, stop=stop, **kw)

    KD = D // 128           # contraction tiles for the projections
    TT = T // 128           # token tiles
    NCH = T // 512          # 512-token chunks
    NS = T // 512           # query strips of 512
    KO = OC // 128          # o-tiles for Q/K (and out-proj contraction)

    nc = bacc.Bacc("TRN2", target_bir_lowering=False)
    xT_d = nc.dram_tensor("xT", [D, T], fp16, kind="ExternalInput")
    wq_d = nc.dram_tensor("wq", [D, OC], fp16, kind="ExternalInput")
    wk_d = nc.dram_tensor("wk", [D, OC], fp16, kind="ExternalInput")
    wv_d = nc.dram_tensor("wv", [D, OC], fp16, kind="ExternalInput")
    bq_d = nc.dram_tensor("bq", [OC], f32, kind="ExternalInput")
    wo_d = nc.dram_tensor("wo", [OC, D], fp16, kind="ExternalInput")
    out_d = nc.dram_tensor("out", [T, D], fp16, kind="ExternalOutput")
    r_dram = nc.dram_tensor("r_scratch", [HC, T], f32)

    with TileContext(nc) as tc:
        with (
            tc.tile_pool(name="persist", bufs=1) as P1,
            tc.tile_pool(name="work", bufs=3) as WK,
            # PSUM budget (8 banks): 2x[128,1024] S^T rotation (4 banks) +
            # 2x[128,512] filler rotation (2 banks) + 2x[128,512] PV /
            # 1/l-broadcast (2 banks; the broadcast reuses the po0 ring).
            tc.tile_pool(name="pss", bufs=2, space="PSUM") as PSS,
            tc.tile_pool(name="psf", bufs=2, space="PSUM") as PSF,
            tc.tile_pool(name="pso", bufs=1, space="PSUM") as PSO,
        ):
            QT = P1.tile([128, KO, T], fp16)
            KT = P1.tile([128, KO, T], fp16)
            V = P1.tile([128, TT, HC, DH + 1], fp16)
            attnT = P1.tile([128, KO, T], fp16)
            wo = P1.tile([128, KO, D], fp16)
            wq = P1.tile([128, KD, OC], fp16)
            wk = P1.tile([128, KD, OC], fp16)
            wv = P1.tile([128, KD, OC], fp16)
            bq = P1.tile([128, KO], f32)
            xT = P1.tile([128, KD, T], fp16)
            OUT = P1.tile([128, TT, D], fp16)

            # DMA priority order: v(0) needs wv + xT chunk 0; the first QK
            # chunk adds wq/wk; later chunks stream behind; wo last. One
            # descriptor per block -- descriptor issue is ~0.6us each on
            # the sync queue and would otherwise gate data arrival.
            nc.sync.dma_start(bq[:], bq_d[:].rearrange("(o p) -> p o", p=128))
            wq_r = wq_d[:].rearrange("(k p) o -> p k o", p=128)
            wk_r = wk_d[:].rearrange("(k p) o -> p k o", p=128)
            wv_r = wv_d[:].rearrange("(k p) o -> p k o", p=128)
            xT_r = xT_d[:].rearrange("(k p) t -> p k t", p=128)
            # chunk 0 + wv are needed first: split them into per-k
            # descriptors so they take a larger share of the DMA engines'
            # descriptor round-robin and finish early
            for k in range(KD):
                nc.sync.dma_start(xT[:, k, 0:512], xT_r[:, k, 0:512])
                nc.sync.dma_start(wv[:, k, :], wv_r[:, k, :])
            nc.sync.dma_start(wq[:], wq_r[:])
            nc.sync.dma_start(wk[:], wk_r[:])
            nc.sync.dma_start(xT[:, :, 512:1024], xT_r[:, :, 512:1024])

            def issue_late_dma(which):
                # issued mid-schedule so early chunks get the full
                # aggregate DMA bandwidth (packets round-robin across all
                # in-flight descriptors)
                if which < 2:
                    ch = 2 + which
                    nc.sync.dma_start(xT[:, :, ch * 512:(ch + 1) * 512],
                                      xT_r[:, :, ch * 512:(ch + 1) * 512])
                else:
                    nc.sync.dma_start(
                        wo[:], wo_d[:].rearrange("(k p) n -> p k n", p=128))

            LP_cm = tc.tile_pool(name="late", bufs=3)
            LP = LP_cm.__enter__()
            ones32 = P1.tile([128, 1], f32)
            nc.gpsimd.memset(ones32[:], 1.0)
            _oap = ones32[:]
            dum = P1.tile([128, 512], fp16)
            nc.gpsimd.memset(dum[:], 0.0625)

            # ---- PE warm-up: dummy matmuls with no DMA dependency ----
            for i in range(NDUMMY):
                psd = PSF.tile([128, 512], f32, tag="fl", name="dmm")
                mm(psd[:], dum[:, 0:128], dum[:], start=True, stop=True)

            # ---- filler generators (yield ~ every 2 matmuls) ----
            def gen_qk(w_t, dst, ot, ch, with_bias):
                ps = PSF.tile([128, 512], f32, tag="fl", name="qkc")
                for k in range(KD):
                    mm(ps[:], w_t[:, k, ot * 128:(ot + 1) * 128],
                       xT[:, k, ch * 512:(ch + 1) * 512],
                       start=(k == 0), stop=(k == KD - 1))
                    if k % 2 == 1 and k < KD - 1:
                        yield
                if with_bias:
                    nc.vector.tensor_scalar_add(
                        dst[:, ot, ch * 512:(ch + 1) * 512], ps[:],
                        bq[:, ot:ot + 1])
                else:
                    nc.vector.tensor_copy(
                        dst[:, ot, ch * 512:(ch + 1) * 512], ps[:])
                yield

            def gen_v(tg):
                for half in range(2):
                    t0 = 4 * tg + 2 * half
                    ps = PSF.tile([128, 512], f32, tag="fl", name="vps")
                    for t4 in range(2):
                        tt = t0 + t4
                        for k in range(KD):
                            mm(ps[:, t4 * 256:(t4 + 1) * 256],
                               xT[:, k, tt * 128:(tt + 1) * 128], wv[:, k, :],
                               start=(k == 0), stop=(k == KD - 1))
                            if k % 2 == 1 and not (t4 == 1 and k == KD - 1):
                                yield
                    nc.vector.tensor_copy(
                        V[:, t0:t0 + 2, :, 0:DH],
                        ps[:].rearrange("p (t h o) -> p t h o", t=2, h=HC))
                    # ones column (memset doesn't accept 16-bit dtypes)
                    nc.vector.tensor_copy(
                        V[:, t0:t0 + 2, :, DH:DH + 1],
                        bass.AP(_oap.tensor, _oap.offset,
                                [_oap.ap[0], [0, 2], [0, HC], [0, 1]]))
                    yield

            def gen_outproj(s):
                for tt in range(4 * s, 4 * s + 4):
                    for nch in range(2):
                        ps = PSF.tile([128, 512], f32, tag="fl", name="ops")
                        for k2 in range(KO):
                            mm(ps[:], attnT[:, k2, tt * 128:(tt + 1) * 128],
                               wo[:, k2, nch * 512:(nch + 1) * 512],
                               start=(k2 == 0), stop=(k2 == KO - 1))
                        nc.vector.tensor_copy(
                            OUT[:, tt, nch * 512:(nch + 1) * 512], ps[:])
                        yield

            def store_outproj(s):
                # one descriptor per 4-tile group, emitted only once the
                # group's CASTs are long done (no sync-queue head-blocking)
                nc.sync.dma_start(
                    out_d[s * 512:(s + 1) * 512, :].rearrange(
                        "(tt p) n -> p tt n", p=128),
                    OUT[:, 4 * s:4 * s + 4, :])

            class Filler:
                def __init__(self):
                    self.q = []

                def add(self, name, gen):
                    self.q.append([name, gen])

                def pump(self, n):
                    while n > 0 and self.q:
                        try:
                            next(self.q[0][1])
                            n -= 1
                        except StopIteration:
                            self.q.pop(0)

                def drain_through(self, name):
                    while any(e[0] == name for e in self.q):
                        try:
                            next(self.q[0][1])
                        except StopIteration:
                            self.q.pop(0)

                def drain_all(self):
                    while self.q:
                        try:
                            next(self.q[0][1])
                        except StopIteration:
                            self.q.pop(0)

            def attn_strip(hp, s, pump):
                heads = (2 * hp, 2 * hp + 1)
                nk = 4 * (s + 1)
                pso = {h: PSO.tile([128, 512], f32, tag=f"po{h % 2}",
                                   name=f"pso{h}")
                       for h in heads}
                for kg in range(nk // 2):    # groups of 2 key tiles
                    kil0 = 2 * kg - (nk - 4)
                    pss = {h: PSS.tile([128, 1024], f32, tag="ss",
                                       name=f"pss{h}")
                           for h in heads}
                    for kk in range(2):
                        ki = 2 * kg + kk
                        qlo = max(0, 128 * (ki - (nk - 4)))
                        for h in heads:
                            po = (h % 2) * 64
                            mm(pss[h][:, kk * 512 + qlo:(kk + 1) * 512],
                               KT[po:po + 64, hp, ki * 128:(ki + 1) * 128],
                               QT[po:po + 64, hp, s * 512 + qlo:(s + 1) * 512],
                               start=True, stop=True)
                    pt = {}
                    for h in heads:
                        pt[h] = LP.tile([128, 1024], fp16,
                                        tag=f"pt{h % 2}", name=f"pt{h}")
                        if kil0 >= 0:
                            # diagonal group: skip the fully-masked q range
                            for kk in range(2):
                                qlo = 128 * (kil0 + kk)
                                nc.scalar.activation(
                                    pt[h][:, kk * 512 + qlo:(kk + 1) * 512],
                                    pss[h][:, kk * 512 + qlo:(kk + 1) * 512],
                                    AF.Exp, scale=0.125)
                        else:
                            nc.scalar.activation(pt[h][:], pss[h][:], AF.Exp,
                                                 scale=0.125)
                    if kil0 >= 0:
                        # within-block triangle mask: keep where q - p >= 0
                        for kk in range(2):
                            c0 = kk * 512 + 128 * (kil0 + kk)
                            for h in heads:
                                nc.gpsimd.affine_select(
                                    pt[h][:, c0:c0 + 128],
                                    pt[h][:, c0:c0 + 128],
                                    pattern=[[1, 128]],
                                    compare_op=ALU.is_ge, fill=0.0,
                                    base=0, channel_multiplier=-1)
                    pump()
                    for kk in range(2):
                        ki = 2 * kg + kk
                        qlo = max(0, 128 * (ki - (nk - 4)))
                        for h in heads:
                            mm(pso[h][0:DH + 1, qlo:512], V[:, ki, h, :],
                               pt[h][:, kk * 512 + qlo:(kk + 1) * 512],
                               start=(ki == 0), stop=(ki == nk - 1),
                               skip_group_check=True)
                # per-strip epilogue: store attn^T, extract l, normalize.
                # 1/l is computed on a [32,16] reshape (16 elems/lane) and
                # partition-broadcast via a DRAM round-trip DMA.
                rb = LP.tile([128, 512], f32, tag="rb")
                for h in heads:
                    po = (h % 2) * 64
                    nc.vector.tensor_copy(
                        attnT[po:po + 64, hp, s * 512:(s + 1) * 512],
                        pso[h][0:DH, :])
                    ls = WK.tile([1, 512], f32, tag="ls")
                    nc.vector.tensor_copy(ls[:], pso[h][DH:DH + 1, :])
                    l4 = WK.tile([32, 16], f32, tag=f"l4{h % 2}",
                                 name=f"l4{h}")
                    nc.sync.dma_start(
                        l4[:], ls[:].rearrange("o (p j) -> o p j", p=32))
                    r4 = WK.tile([32, 16], f32, tag=f"r4{h % 2}",
                                 name=f"r4{h}")
                    nc.vector.reciprocal(r4[:], l4[:])
                    nc.sync.dma_start(
                        r_dram[h:h + 1, s * 512:(s + 1) * 512], r4[:])
                    nc.sync.dma_start(
                        rb[po:po + 64, :],
                        bass.AP(r_dram, h * T + s * 512,
                                [[0, 64], [1, 512]]))
                nc.vector.tensor_mul(
                    attnT[:, hp, s * 512:(s + 1) * 512],
                    attnT[:, hp, s * 512:(s + 1) * 512], rb[:])

            # ---- emission schedule ----
            # strip (0,0) prerequisites run inline; everything else is
            # queued as filler and pumped between attention key-groups.
            for _ in gen_v(0):
                pass
            for _ in gen_qk(wk, KT, 0, 0, False):
                pass
            for _ in gen_qk(wq, QT, 0, 0, True):
                pass

            F = Filler()
            F.add('k01', gen_qk(wk, KT, 0, 1, False))
            F.add('q01', gen_qk(wq, QT, 0, 1, True))
            F.add('v1', gen_v(1))
            F.add('k10', gen_qk(wk, KT, 1, 0, False))
            F.add('q10', gen_qk(wq, QT, 1, 0, True))
            F.add('k02', gen_qk(wk, KT, 0, 2, False))
            F.add('q02', gen_qk(wq, QT, 0, 2, True))
            F.add('v2', gen_v(2))
            F.add('k11', gen_qk(wk, KT, 1, 1, False))
            F.add('q11', gen_qk(wq, QT, 1, 1, True))
            F.add('k03', gen_qk(wk, KT, 0, 3, False))
            F.add('q03', gen_qk(wq, QT, 0, 3, True))
            F.add('v3', gen_v(3))
            F.add('k12', gen_qk(wk, KT, 1, 2, False))
            F.add('q12', gen_qk(wq, QT, 1, 2, True))
            F.add('k13', gen_qk(wk, KT, 1, 3, False))
            F.add('q13', gen_qk(wq, QT, 1, 3, True))

            def pump():
                F.pump(PUMP_N)

            for s in range(NS):
                if s >= 1:
                    F.drain_through(f'v{s}')
                if s < 2:
                    issue_late_dma(s)       # xT chunk 2+s
                if s == 2:
                    issue_late_dma(2)       # wo
                pump()
                attn_strip(0, s, pump)
            for s in range(NS):
                F.drain_through(f'q1{s}')
                pump()
                attn_strip(1, s, pump)
                F.add(f'op{s}', gen_outproj(s))
                if s >= 2:
                    # two strips of pump time have passed; this is a no-op
                    # drain in the common case
                    F.drain_through(f'op{s - 2}')
                    store_outproj(s - 2)
            F.drain_through('op2')
            store_outproj(2)
            F.drain_all()
            store_outproj(3)
            LP_cm.__exit__(None, None, None)

    if hasattr(nc, "compile"):
        nc.compile()
    return nc


def shard_inputs(x, w_qkv, b_qkv, w_out):
    """Build the 8 per-core input dicts (core = b * 4 + g)."""
    in_maps = []
    for core in range(NCORES):
        b, g = core // 4, core % 4
        o0 = g * OC
        in_maps.append({
            "xT": np.ascontiguousarray(np.asarray(x[b]).T.astype(np.float16)),
            "wq": np.ascontiguousarray(w_qkv[:, o0:o0 + OC].astype(np.float16)),
            "wk": np.ascontiguousarray(w_qkv[:, D + o0:D + o0 + OC].astype(np.float16)),
            "wv": np.ascontiguousarray(w_qkv[:, 2 * D + o0:2 * D + o0 + OC].astype(np.float16)),
            "bq": np.ascontiguousarray(b_qkv[o0:o0 + OC].astype(np.float32)),
            "wo": np.ascontiguousarray(w_out[o0:o0 + OC, :].astype(np.float16)),
        })
    return in_maps


_NC_CACHE = {}


def kernel(x, w_qkv, b_qkv, w_out, b_out):
    from concourse.bass_utils import run_bass_kernel_spmd

    x = np.asarray(x, dtype=np.float32)
    w_qkv = np.asarray(w_qkv, dtype=np.float32)
    b_qkv = np.asarray(b_qkv, dtype=np.float32)
    w_out = np.asarray(w_out, dtype=np.float32)
    b_out = np.asarray(b_out, dtype=np.float32)

    if "nc" not in _NC_CACHE:
        _NC_CACHE["nc"] = build_nc(T_FULL)
    nc = _NC_CACHE["nc"]

    in_maps = shard_inputs(x, w_qkv, b_qkv, w_out)
    res = run_bass_kernel_spmd(nc, in_maps, list(range(NCORES)))

    # b_v and b_out folded here: softmax rows sum to 1, so the v-bias
    # contributes b_v @ w_out to every token.
    b_eff = (b_out + b_qkv[2 * D:] @ w_out).astype(np.float32)
    out = np.empty((B, T_FULL, D), dtype=np.float32)
    for b in range(B):
        acc = res.results[b * 4]["out"].astype(np.float32)
        for g in range(1, 4):
            acc = acc + res.results[b * 4 + g]["out"].astype(np.float32)
        out[b] = acc + b_eff
    return out


# revision 37
# speedup vs baseline: 1.0192x; 1.0192x over previous
"""Causal self-attention (B=2, T=2048, D=1024, H=16) on 8 TRN2 NeuronCores.

Sharding: data-parallel over batch (2) x tensor-parallel over head groups (4),
so each core handles one batch element and 4 heads (256 of the 1024 attention
channels). The out-projection is row-sharded; the host sums the 4 partial
outputs per batch element in fp32.

v3 schedule (v1 baseline ~190us, v2 ~234us):
  - one consolidated DMA descriptor per logical input block (9 total) in
    token-chunk-major order, so the DMA-issue queue (~0.6us per descriptor)
    never gates data arrival; packets of one descriptor spread across all
    16 DMA engines
  - ~12 dummy matmuls at t=0 keep the PE busy through the DMA fill so the
    HAM clock-gate reaches K=8/8 (2.4 GHz) before real work starts
  - all projection work not needed up front is wrapped in generators and
    pumped between the S^T->exp->PV stages of each attention key-group, so
    the PE always has independent work while ScalarE (exp, the attention
    pacer at ~2.25us/key-group) runs; out-projection tiles are pumped the
    same way during the second head-pair's strips instead of as a tail
  - causal narrowing: S^T, exp and PV skip the fully-masked query range of
    diagonal key tiles (query quantization 128); only the within-block
    triangle is masked, with a [128,128] affine_select per diagonal block
  - softmax denominator comes free as PV row 64 (lhsT = [V | 1]); the
    1/l broadcast across the 64 output channels is a K=1 PE matmul into a
    reused PSUM bank + one DVE reciprocal -- no DRAM round trip
Bias handling: b_k dropped (softmax shift-invariant per query), b_q applied
via a DVE per-partition scalar add, b_v and b_out folded into a host-side
constant (softmax rows sum to 1).
"""

import numpy as np

B, T_FULL, D, H = 2, 2048, 1024, 16
DH = 64
HC = 4            # heads per core
OC = HC * DH      # 256 attention channels per core
NCORES = 8

NDUMMY = 8        # PE warm-up matmuls during the initial DMA fill
PUMP_N = 4        # filler units (~2 matmuls each) emitted per key-group


def build_nc(T=T_FULL):
    import concourse.bass as bass
    import concourse.mybir as mybir
    from concourse import bacc
    from concourse.tile import TileContext

    f32 = mybir.dt.float32
    f32r = mybir.dt.float32r
    fp16 = mybir.dt.float16
    AF = mybir.ActivationFunctionType
    ALU = mybir.AluOpType

    def mm(out, lhsT, rhs, start, stop, **kw):
        if lhsT.dtype == f32:
            lhsT = lhsT.bitcast(f32r)
        if rhs.dtype == f32:
            rhs = rhs.bitcast(f32r)
        nc.tensor.matmul(out, lhsT, rhs, start=start, stop=stop, **kw)

    KD = D // 128           # contraction tiles for the projections
    TT = T // 128           # token tiles
    NCH = T // 512          # 512-token chunks
    NS = T // 512           # query strips of 512
    KO = OC // 128          # o-tiles for Q/K (and out-proj contraction)

    nc = bacc.Bacc("TRN2", target_bir_lowering=False)
    xT_d = nc.dram_tensor("xT", [D, T], fp16, kind="ExternalInput")
    wq_d = nc.dram_tensor("wq", [D, OC], fp16, kind="ExternalInput")
    wk_d = nc.dram_tensor("wk", [D, OC], fp16, kind="ExternalInput")
    wv_d = nc.dram_tensor("wv", [D, OC], fp16, kind="ExternalInput")
    bq_d = nc.dram_tensor("bq", [OC], f32, kind="ExternalInput")
    wo_d = nc.dram_tensor("wo", [OC, D], fp16, kind="ExternalInput")
    out_d = nc.dram_tensor("out", [T, D], fp16, kind="ExternalOutput")
    r_dram = nc.dram_tensor("r_scratch", [HC, T], f32)

    with TileContext(nc) as tc:
        with (
            tc.tile_pool(name="persist", bufs=1) as P1,
            tc.tile_pool(name="work", bufs=3) as WK,
            # PSUM budget (8 banks): 2x[128,1024] S^T rotation (4 banks) +
            # 2x[128,512] filler rotation (2 banks) + 2x[128,512] PV /
            # 1/l-broadcast (2 banks; the broadcast reuses the po0 ring).
            tc.tile_pool(name="pss", bufs=2, space="PSUM") as PSS,
            tc.tile_pool(name="psf", bufs=2, space="PSUM") as PSF,
            tc.tile_pool(name="pso", bufs=1, space="PSUM") as PSO,
        ):
            QT = P1.tile([128, KO, T], fp16)
            KT = P1.tile([128, KO, T], fp16)
            V = P1.tile([128, TT, HC, DH + 1], fp16)
            attnT = P1.tile([128, KO, T], fp16)
            wo = P1.tile([128, KO, D], fp16)
            wq = P1.tile([128, KD, OC], fp16)
            wk = P1.tile([128, KD, OC], fp16)
            wv = P1.tile([128, KD, OC], fp16)
            bq = P1.tile([128, KO], f32)
            xT = P1.tile([128, KD, T], fp16)
            OUT = P1.tile([128, TT, D], fp16)

            # DMA priority order: v(0) needs wv + xT chunk 0; the first QK
            # chunk adds wq/wk; later chunks stream behind; wo last. One
            # descriptor per block -- descriptor issue is ~0.6us each on
            # the sync queue and would otherwise gate data arrival.
            nc.sync.dma_start(bq[:], bq_d[:].rearrange("(o p) -> p o", p=128))
            wq_r = wq_d[:].rearrange("(k p) o -> p k o", p=128)
            wk_r = wk_d[:].rearrange("(k p) o -> p k o", p=128)
            wv_r = wv_d[:].rearrange("(k p) o -> p k o", p=128)
            xT_r = xT_d[:].rearrange("(k p) t -> p k t", p=128)
            # chunk 0 is needed first: split into per-k descriptors so it
            # takes a larger share of the DMA engines' descriptor
            # round-robin and finishes early
            for k in range(KD):
                nc.sync.dma_start(xT[:, k, 0:512], xT_r[:, k, 0:512])
            nc.sync.dma_start(wk[:], wk_r[:])
            nc.sync.dma_start(wq[:], wq_r[:])
            nc.sync.dma_start(wv[:], wv_r[:])
            nc.sync.dma_start(xT[:, :, 512:1024], xT_r[:, :, 512:1024])

            def issue_late_dma(which):
                # issued mid-schedule so early chunks get the full
                # aggregate DMA bandwidth (packets round-robin across all
                # in-flight descriptors)
                if which < 2:
                    ch = 2 + which
                    nc.sync.dma_start(xT[:, :, ch * 512:(ch + 1) * 512],
                                      xT_r[:, :, ch * 512:(ch + 1) * 512])
                else:
                    nc.sync.dma_start(
                        wo[:], wo_d[:].rearrange("(k p) n -> p k n", p=128))

            LP_cm = tc.tile_pool(name="late", bufs=3)
            LP = LP_cm.__enter__()
            ones32 = P1.tile([128, 1], f32)
            nc.gpsimd.memset(ones32[:], 1.0)
            _oap = ones32[:]
            ones64 = P1.tile([1, DH], fp16)
            nc.vector.tensor_copy(
                ones64[:],
                bass.AP(_oap.tensor, _oap.offset, [[_oap.ap[0][0], 1], [0, DH]]))
            dum = P1.tile([128, 512], fp16)
            nc.gpsimd.memset(dum[:], 0.0625)

            # ---- PE warm-up: dummy matmuls with no DMA dependency ----
            for i in range(NDUMMY):
                psd = PSF.tile([128, 512], f32, tag="fl", name="dmm")
                mm(psd[:], dum[:, 0:128], dum[:], start=True, stop=True)

            # ---- filler generators (yield ~ every 2 matmuls) ----
            def gen_qk(w_t, dst, ot, ch, with_bias):
                ps = PSF.tile([128, 512], f32, tag="fl", name="qkc")
                for k in range(KD):
                    mm(ps[:], w_t[:, k, ot * 128:(ot + 1) * 128],
                       xT[:, k, ch * 512:(ch + 1) * 512],
                       start=(k == 0), stop=(k == KD - 1))
                    if k % 2 == 1 and k < KD - 1:
                        yield
                if with_bias:
                    nc.vector.tensor_scalar_add(
                        dst[:, ot, ch * 512:(ch + 1) * 512], ps[:],
                        bq[:, ot:ot + 1])
                else:
                    nc.vector.tensor_copy(
                        dst[:, ot, ch * 512:(ch + 1) * 512], ps[:])
                yield

            def gen_v_half(tg, half):
                t0 = 4 * tg + 2 * half
                ps = PSF.tile([128, 512], f32, tag="fl", name="vps")
                for t4 in range(2):
                    tt = t0 + t4
                    for k in range(KD):
                        mm(ps[:, t4 * 256:(t4 + 1) * 256],
                           xT[:, k, tt * 128:(tt + 1) * 128], wv[:, k, :],
                           start=(k == 0), stop=(k == KD - 1))
                        if k % 2 == 1 and not (t4 == 1 and k == KD - 1):
                            yield
                nc.vector.tensor_copy(
                    V[:, t0:t0 + 2, :, 0:DH],
                    ps[:].rearrange("p (t h o) -> p t h o", t=2, h=HC))
                # ones column (memset doesn't accept 16-bit dtypes)
                nc.vector.tensor_copy(
                    V[:, t0:t0 + 2, :, DH:DH + 1],
                    bass.AP(_oap.tensor, _oap.offset,
                            [_oap.ap[0], [0, 2], [0, HC], [0, 1]]))
                yield

            def gen_outproj(s):
                for tt in range(4 * s, 4 * s + 4):
                    for nch in range(2):
                        ps = PSF.tile([128, 512], f32, tag="fl", name="ops")
                        for k2 in range(KO):
                            mm(ps[:], attnT[:, k2, tt * 128:(tt + 1) * 128],
                               wo[:, k2, nch * 512:(nch + 1) * 512],
                               start=(k2 == 0), stop=(k2 == KO - 1))
                        nc.vector.tensor_copy(
                            OUT[:, tt, nch * 512:(nch + 1) * 512], ps[:])
                        yield

            def store_outproj(s):
                # one descriptor per 4-tile group, emitted only once the
                # group's CASTs are long done (no sync-queue head-blocking)
                nc.sync.dma_start(
                    out_d[s * 512:(s + 1) * 512, :].rearrange(
                        "(tt p) n -> p tt n", p=128),
                    OUT[:, 4 * s:4 * s + 4, :])

            class Filler:
                def __init__(self):
                    self.q = []

                def add(self, name, gen):
                    self.q.append([name, gen])

                def pump(self, n):
                    while n > 0 and self.q:
                        try:
                            next(self.q[0][1])
                            n -= 1
                        except StopIteration:
                            self.q.pop(0)

                def drain_through(self, name):
                    while any(e[0] == name for e in self.q):
                        try:
                            next(self.q[0][1])
                        except StopIteration:
                            self.q.pop(0)

                def drain_all(self):
                    while self.q:
                        try:
                            next(self.q[0][1])
                        except StopIteration:
                            self.q.pop(0)

            def attn_strip(hp, s, pump, prereq=None, fast_norm=False):
                heads = (2 * hp, 2 * hp + 1)
                nk = 4 * (s + 1)
                pso = {h: PSO.tile([128, 512], f32, tag=f"po{h % 2}",
                                   name=f"pso{h}")
                       for h in heads}
                for kg in range(nk // 2):    # groups of 2 key tiles
                    kil0 = 2 * kg - (nk - 4)
                    pss = {h: PSS.tile([128, 1024], f32, tag="ss",
                                       name=f"pss{h}")
                           for h in heads}
                    for kk in range(2):
                        ki = 2 * kg + kk
                        qlo = max(0, 128 * (ki - (nk - 4)))
                        for h in heads:
                            po = (h % 2) * 64
                            mm(pss[h][:, kk * 512 + qlo:(kk + 1) * 512],
                               KT[po:po + 64, hp, ki * 128:(ki + 1) * 128],
                               QT[po:po + 64, hp, s * 512 + qlo:(s + 1) * 512],
                               start=True, stop=True)
                    pt = {}
                    for h in heads:
                        pt[h] = LP.tile([128, 1024], fp16,
                                        tag=f"pt{h % 2}", name=f"pt{h}")
                        if kil0 >= 0:
                            # diagonal group: skip the fully-masked q range
                            for kk in range(2):
                                qlo = 128 * (kil0 + kk)
                                nc.scalar.activation(
                                    pt[h][:, kk * 512 + qlo:(kk + 1) * 512],
                                    pss[h][:, kk * 512 + qlo:(kk + 1) * 512],
                                    AF.Exp, scale=0.125)
                        else:
                            nc.scalar.activation(pt[h][:], pss[h][:], AF.Exp,
                                                 scale=0.125)
                    if kil0 >= 0:
                        # within-block triangle mask: keep where q - p >= 0
                        for kk in range(2):
                            c0 = kk * 512 + 128 * (kil0 + kk)
                            for h in heads:
                                nc.gpsimd.affine_select(
                                    pt[h][:, c0:c0 + 128],
                                    pt[h][:, c0:c0 + 128],
                                    pattern=[[1, 128]],
                                    compare_op=ALU.is_ge, fill=0.0,
                                    base=0, channel_multiplier=-1)
                    if prereq is not None:
                        # emit this key-group's data prerequisites here, so
                        # the burst lands between exp and PV where the PE
                        # would otherwise idle
                        prereq(kg)
                    pump()
                    for kk in range(2):
                        ki = 2 * kg + kk
                        qlo = max(0, 128 * (ki - (nk - 4)))
                        for h in heads:
                            mm(pso[h][0:DH + 1, qlo:512], V[:, ki, h, :],
                               pt[h][:, kk * 512 + qlo:(kk + 1) * 512],
                               start=(ki == 0), stop=(ki == nk - 1),
                               skip_group_check=True)
                # per-strip epilogue: store attn^T, extract l, normalize.
                if fast_norm:
                    # used for the final strip only, where the DMA
                    # round-trip latency would sit on the critical path:
                    # single-pass DVE reciprocal + K=1 PE broadcast.
                    lr = {}
                    for h in heads:
                        po = (h % 2) * 64
                        nc.vector.tensor_copy(
                            attnT[po:po + 64, hp, s * 512:(s + 1) * 512],
                            pso[h][0:DH, :])
                        ls = WK.tile([1, 512], f32, tag=f"lf{h % 2}",
                                     name=f"lf{h}")
                        nc.vector.tensor_copy(ls[:], pso[h][DH:DH + 1, :])
                        lrf = WK.tile([1, 512], f32, tag=f"lrf{h % 2}",
                                      name=f"lrf{h}")
                        nc.vector.reciprocal_approx_fast(lrf[:], ls[:])
                        lr[h] = WK.tile([1, 512], fp16, tag=f"lr{h % 2}",
                                        name=f"lr{h}")
                        nc.vector.tensor_copy(lr[h][:], lrf[:])
                    rbq = PSO.tile([128, 512], f32, tag="po0", name="bc")
                    for h in heads:
                        po = (h % 2) * 64
                        nc.tensor.matmul(rbq[po:po + 64, :], ones64[:],
                                         lr[h][:], start=True, stop=True)
                    nc.vector.tensor_mul(
                        attnT[:, hp, s * 512:(s + 1) * 512],
                        attnT[:, hp, s * 512:(s + 1) * 512], rbq[:])
                    return
                # 1/l on a [32,16] reshape (16 elems/lane) and partition-
                # broadcast via a DRAM round-trip DMA; the latency hides
                # under other strips' work.
                rb = LP.tile([128, 512], f32, tag="rb")
                for h in heads:
                    po = (h % 2) * 64
                    nc.vector.tensor_copy(
                        attnT[po:po + 64, hp, s * 512:(s + 1) * 512],
                        pso[h][0:DH, :])
                    ls = WK.tile([1, 512], f32, tag="ls")
                    nc.vector.tensor_copy(ls[:], pso[h][DH:DH + 1, :])
                    l4 = WK.tile([32, 16], f32, tag=f"l4{h % 2}",
                                 name=f"l4{h}")
                    nc.sync.dma_start(
                        l4[:], ls[:].rearrange("o (p j) -> o p j", p=32))
                    r4 = WK.tile([32, 16], f32, tag=f"r4{h % 2}",
                                 name=f"r4{h}")
                    nc.vector.reciprocal(r4[:], l4[:])
                    nc.sync.dma_start(
                        r_dram[h:h + 1, s * 512:(s + 1) * 512], r4[:])
                    nc.sync.dma_start(
                        rb[po:po + 64, :],
                        bass.AP(r_dram, h * T + s * 512,
                                [[0, 64], [1, 512]]))
                nc.vector.tensor_mul(
                    attnT[:, hp, s * 512:(s + 1) * 512],
                    attnT[:, hp, s * 512:(s + 1) * 512], rb[:])

            # ---- emission schedule ----
            # only the strip-(0,0) QK chunk runs inline; V(0) and everything
            # else is filler, pumped between attention key-groups (the V
            # tiles a key-group needs are force-drained right between its
            # exp and PV, where the PE would otherwise idle).
            for _ in gen_qk(wk, KT, 0, 0, False):
                pass
            for _ in gen_qk(wq, QT, 0, 0, True):
                pass

            F = Filler()
            F.add('v0h0', gen_v_half(0, 0))
            F.add('v0h1', gen_v_half(0, 1))
            F.add('k01', gen_qk(wk, KT, 0, 1, False))
            F.add('q01', gen_qk(wq, QT, 0, 1, True))
            F.add('v1h0', gen_v_half(1, 0))
            F.add('v1h1', gen_v_half(1, 1))
            F.add('k10', gen_qk(wk, KT, 1, 0, False))
            F.add('q10', gen_qk(wq, QT, 1, 0, True))
            F.add('k02', gen_qk(wk, KT, 0, 2, False))
            F.add('q02', gen_qk(wq, QT, 0, 2, True))
            F.add('v2h0', gen_v_half(2, 0))
            F.add('v2h1', gen_v_half(2, 1))
            F.add('k11', gen_qk(wk, KT, 1, 1, False))
            F.add('q11', gen_qk(wq, QT, 1, 1, True))
            F.add('k03', gen_qk(wk, KT, 0, 3, False))
            F.add('q03', gen_qk(wq, QT, 0, 3, True))
            F.add('v3h0', gen_v_half(3, 0))
            F.add('v3h1', gen_v_half(3, 1))
            F.add('k12', gen_qk(wk, KT, 1, 2, False))
            F.add('q12', gen_qk(wq, QT, 1, 2, True))
            F.add('k13', gen_qk(wk, KT, 1, 3, False))
            F.add('q13', gen_qk(wq, QT, 1, 3, True))

            def pump():
                F.pump(PUMP_N)

            def hp0_prereq(kg):
                # this key-group's PV reads V tiles 2*kg, 2*kg+1, which
                # live in chunk kg//2, half kg%2
                F.drain_through(f'v{kg // 2}h{kg % 2}')

            for s in range(NS):
                if s < 2:
                    issue_late_dma(s)       # xT chunk 2+s
                if s == 2:
                    issue_late_dma(2)       # wo
                if s >= 1:
                    # this strip's S^T reads QT/KT chunk s
                    F.drain_through(f'q0{s}')
                pump()
                attn_strip(0, s, pump, prereq=hp0_prereq)
            for s in range(NS):
                F.drain_through(f'q1{s}')
                pump()
                attn_strip(1, s, pump, fast_norm=(s == NS - 1))
                F.add(f'op{s}', gen_outproj(s))
                if s >= 2:
                    # two strips of pump time have passed; this is a no-op
                    # drain in the common case
                    F.drain_through(f'op{s - 2}')
                    store_outproj(s - 2)
            F.drain_through('op2')
            store_outproj(2)
            F.drain_all()
            store_outproj(3)
            LP_cm.__exit__(None, None, None)

    if hasattr(nc, "compile"):
        nc.compile()
    return nc


def shard_inputs(x, w_qkv, b_qkv, w_out):
    """Build the 8 per-core input dicts (core = b * 4 + g)."""
    in_maps = []
    for core in range(NCORES):
        b, g = core // 4, core % 4
        o0 = g * OC
        in_maps.append({
            "xT": np.ascontiguousarray(np.asarray(x[b]).T.astype(np.float16)),
            "wq": np.ascontiguousarray(w_qkv[:, o0:o0 + OC].astype(np.float16)),
            "wk": np.ascontiguousarray(w_qkv[:, D + o0:D + o0 + OC].astype(np.float16)),
            "wv": np.ascontiguousarray(w_qkv[:, 2 * D + o0:2 * D + o0 + OC].astype(np.float16)),
            "bq": np.ascontiguousarray(b_qkv[o0:o0 + OC].astype(np.float32)),
            "wo": np.ascontiguousarray(w_out[o0:o0 + OC, :].astype(np.float16)),
        })
    return in_maps


_NC_CACHE = {}


def kernel(x, w_qkv, b_qkv, w_out, b_out):
    from concourse.bass_utils import run_bass_kernel_spmd

    x = np.asarray(x, dtype=np.float32)
    w_qkv = np.asarray(w_qkv, dtype=np.float32)
    b_qkv = np.asarray(b_qkv, dtype=np.float32)
    w_out = np.asarray(w_out, dtype=np.float32)
    b_out = np.asarray(b_out, dtype=np.float32)

    if "nc" not in _NC_CACHE:
        _NC_CACHE["nc"] = build_nc(T_FULL)
    nc = _NC_CACHE["nc"]

    in_maps = shard_inputs(x, w_qkv, b_qkv, w_out)
    res = run_bass_kernel_spmd(nc, in_maps, list(range(NCORES)))

    # b_v and b_out folded here: softmax rows sum to 1, so the v-bias
    # contributes b_v @ w_out to every token.
    b_eff = (b_out + b_qkv[2 * D:] @ w_out).astype(np.float32)
    out = np.empty((B, T_FULL, D), dtype=np.float32)
    for b in range(B):
        acc = res.results[b * 4]["out"].astype(np.float32)
        for g in range(1, 4):
            acc = acc + res.results[b * 4 + g]["out"].astype(np.float32)
        out[b] = acc + b_eff
    return out
